# revision 6
# baseline (speedup 1.0000x reference)
"""Bengio-style NNLM forward on 8 Trainium2 NeuronCores (Bass/Tile).

Strategy (vocab-tensor-parallel + fp8 DoubleRow matmuls):
  - W_2/b_2 and the [B, VOCAB] output are sharded across 8 cores along
    vocab (6284 columns each, padded from 50257). The small shared parts
    (embedding rows, fc weights) are replicated.
  - All matmuls run in fp8 e4m3 with MatmulPerfMode.DoubleRow (2 K-rows
    per PE cell, 0.5 cycles/row -> 4x over f32r), f32 PSUM accumulation.
  - Precision via mean compensation: device computes with DEVIATIONS
    d = x - x0 (x0 = host reference point), which are ~6x smaller than x,
    so fp8 quantization noise shrinks proportionally. The exact term
    c = x0 @ W2 + b2 is folded in as 3 constant power-of-2 feature rows
    whose fp8 weights are a greedy residual split of c (error < 2e-6).
      d_e  = fp8(2^12 (x_e - xbar_e))    gathered embeddings, host-side
      h    = tanh(2^-24 (d_e @ W1') + b1')  on ACT, b1' = b1 + xbar_e@W1
      d_h  = fp8(2^12 h - 2^12 x0_h)     second ACT pass (Identity+bias)
      out  = 2^-18 (d @ W2' + c-rows)    bf16 out, scaled copy DVE/ACT
  - fp8 bytes travel as uint8 DRAM params and are bitcast on device.
"""
import json
import numpy as np
import ml_dtypes

import concourse.bass as bass
import concourse.mybir as mybir
import concourse.tile as tile
from concourse import bass_utils, bass2jax

F32 = mybir.dt.float32
F8 = mybir.dt.float8e4
U8 = mybir.dt.uint8
BF16 = mybir.dt.bfloat16
F8NP = ml_dtypes.float8_e4m3
BF16NP = ml_dtypes.bfloat16
DR = mybir.MatmulPerfMode.DoubleRow

EMB = 128
CTX = 8
HID = 1000
VOCAB = 50257
B = 4096
N_CORES = 8
VC = 6284            # per-core vocab shard (8 * 6284 = 50272 >= 50257)
KE, KH = 8, 8        # emb / hidden k-tiles (128 features each)
K2 = (KE + KH) * 128 # 2048 layer-2 contraction

SD = 2.0 ** 12       # deviation scale (d_e, d_h)
TW = 2.0 ** 6        # W2 scale; psum = 2^18 * out
FINV = 2.0 ** -18
L1INV = 2.0 ** -24   # layer-1 psum descale (2^12 * 2^12)
CVALS = (2.0 ** 7, 2.0 ** 3, 2.0 ** -1)   # const-row feature values


# ---------------------------------------------------------------------------
# Workaround for walrus builds that allow only ONE sem-wait per instruction:
# split any multi-wait instruction into wait-only EventSemaphores before it.
# ---------------------------------------------------------------------------
def _split_waits(js):
    for fn in js.get("functions", []):
        for bb in fn.get("blocks", []):
            out = []
            for inst in bb.get("instructions", []):
                si = inst.get("sync_info")
                waits = (si or {}).get("on_wait") or []
                if len(waits) > 1:
                    for k, w in enumerate(waits[:-1]):
                        out.append({
                            "debug": inst.get("debug", 0),
                            "engine": inst["engine"],
                            "ins": [], "outs": [],
                            "name": f"{inst['name']}-wsplit{k}",
                            "opcode": "EventSemaphore",
                            "sync_info": {"on_update": [], "on_wait": [w]},
                        })
                    si["on_wait"] = waits[-1:]
                out.append(inst)
            bb["instructions"] = out
    return js


def _install_patches():
    if getattr(bass_utils.compile_bir_kernel, "_wait_split_patched", False):
        return
    orig = bass_utils.compile_bir_kernel

    def wrapper(bir_json, tmpdir, neff_name="file.neff"):
        js = _split_waits(json.loads(bir_json))
        return orig(json.dumps(js).encode(), tmpdir, neff_name=neff_name)

    wrapper._wait_split_patched = True
    bass_utils.compile_bir_kernel = wrapper
    bass2jax.compile_bir_kernel = wrapper


def _fp8(x):
    return np.asarray(x, np.float32).astype(F8NP)


def _fp8f(x):
    return _fp8(x).astype(np.float64)


def _n_chunks(vc, nmax=512):
    out, o = [], 0
    while o < vc:
        w = min(nmax, vc - o)
        out.append((o, w))
        o += w
    return out


def _build_kernel(reps=1, opts=None):
    o = {"psA": 4, "psB": 4, "htp": 2, "w2p": 2, "outp": 4,
         "dh_dve": False, "copy_split": 2, "emb_split": 1,
         "skip_l1": False, "sparse_copy": False, "ht_bf16": True,
         "dma_merge": 4, "dr": False}
    o.update(opts or {})
    nc = bass.Bass()
    embT_d = nc.declare_dram_parameter("embT", [128, KE, B], U8, isOutput=False)
    w1T_d = nc.declare_dram_parameter("w1T", [128, KE, 1024], U8, isOutput=False)
    b1c_d = nc.declare_dram_parameter("b1c", [128, KH], F32, isOutput=False)
    x0c_d = nc.declare_dram_parameter("x0c", [128, KH], F32, isOutput=False)
    w2T_d = nc.declare_dram_parameter("w2T", [128, KE + KH, VC], U8, isOutput=False)
    out_d = nc.declare_dram_parameter("out", [B, VC], BF16, isOutput=True)

    with tile.TileContext(nc) as tc:
        with tc.tile_pool(name="constp", bufs=1) as constp, \
             tc.tile_pool(name="embp", bufs=2) as embp, \
             tc.tile_pool(name="hp", bufs=1) as hp, \
             tc.tile_pool(name="htp", bufs=o["htp"]) as htp, \
             tc.tile_pool(name="w2p", bufs=o["w2p"]) as w2p, \
             tc.tile_pool(name="outp", bufs=o["outp"]) as outp, \
             tc.tile_pool(name="psA", bufs=o["psA"], space="PSUM") as psA, \
             tc.tile_pool(name="psB", bufs=o["psB"], space="PSUM") as psB:

            w1 = constp.tile([128, KE, 1024], F8)
            nc.sync.dma_start(w1[:], w1T_d[:].bitcast(F8))
            b1 = constp.tile([128, KH], F32)
            nc.sync.dma_start(b1[:], b1c_d[:])
            x0 = constp.tile([128, KH], F32)
            nc.sync.dma_start(x0[:], x0c_d[:])

            for _r in range(reps):
                emb = embp.tile([128, KE, B], F8, tag="emb")
                nsp = o["emb_split"]
                for sp in range(nsp):
                    es = slice(sp * (B // nsp), (sp + 1) * (B // nsp))
                    nc.sync.dma_start(emb[:, :, es], embT_d[:, :, es].bitcast(F8))
                dh = emb if o["skip_l1"] else hp.tile([128, KH, B], F8, tag="dh")

                # ---- layer 1: h = tanh(2^-24 ps + b1'); dh = 2^12 h - 2^12 x0_h
                for jt in range(0 if o["skip_l1"] else KH):
                    js = slice(jt * 128, (jt + 1) * 128)
                    for bh in range(B // 512):
                        bs = slice(bh * 512, (bh + 1) * 512)
                        ps = psA.tile([128, 512], F32, tag="psA")
                        if o["dr"]:
                            for k in range(0, KE, 2):
                                nc.tensor.matmul(
                                    ps[:], w1[:, k:k + 2, js],
                                    emb[:, k:k + 2, bs],
                                    start=(k == 0), stop=(k == KE - 2),
                                    perf_mode=DR)
                        else:
                            for k in range(KE):
                                nc.tensor.matmul(
                                    ps[:], w1[:, k, js], emb[:, k, bs],
                                    start=(k == 0), stop=(k == KE - 1))
                        ht = htp.tile([128, 512], BF16 if o["ht_bf16"] else F32,
                                      tag="ht")
                        nc.scalar.activation(
                            ht[:], ps[:], mybir.ActivationFunctionType.Tanh,
                            bias=b1[:, jt:jt + 1], scale=L1INV)
                        if o["dh_dve"]:
                            nc.vector.tensor_scalar(
                                dh[:, jt, bs], ht[:], SD, x0[:, jt:jt + 1],
                                mybir.AluOpType.mult, mybir.AluOpType.add)
                        else:
                            nc.scalar.activation(
                                dh[:, jt, bs], ht[:],
                                mybir.ActivationFunctionType.Identity,
                                bias=x0[:, jt:jt + 1], scale=SD)

                # ---- layer 2: out = 2^-18 (d @ W2' + c-rows)
                for (nb, nw) in _n_chunks(VC):
                    w2 = w2p.tile([128, KE + KH, 512], F8, tag="w2")
                    nc.sync.dma_start(
                        w2[:, :, :nw], w2T_d[:, :, nb:nb + nw].bitcast(F8))
                    for m in range(B // 128):
                        ms = slice(m * 128, (m + 1) * 128)
                        ps = psB.tile([128, 512], F32, tag="psB")
                        if o["dr"]:
                            for k in range(0, KE + KH, 2):
                                lhsT = (emb[:, k:k + 2, ms] if k < KE
                                        else dh[:, k - KE:k - KE + 2, ms])
                                nc.tensor.matmul(
                                    ps[:, :nw], lhsT, w2[:, k:k + 2, :nw],
                                    start=(k == 0), stop=(k == KE + KH - 2),
                                    perf_mode=DR)
                        else:
                            for k in range(KE + KH):
                                lhsT = (emb[:, k, ms] if k < KE
                                        else dh[:, k - KE, ms])
                                nc.tensor.matmul(
                                    ps[:, :nw], lhsT, w2[:, k, :nw],
                                    start=(k == 0), stop=(k == KE + KH - 1))
                        if o["sparse_copy"] and m % 8 != 7:
                            continue
                        dm = o["dma_merge"]
                        mi = m % dm
                        if mi == 0:
                            o_t = outp.tile([128, dm, 512], BF16, tag="o")
                        cs = o["copy_split"]
                        if (cs == 0 or (cs == 2 and m % 2 == 0)
                                or (cs == 4 and m % 3 < 2)):
                            nc.vector.tensor_scalar_mul(
                                o_t[:, mi, :nw], ps[:, :nw], FINV)
                        else:
                            nc.scalar.activation(
                                o_t[:, mi, :nw], ps[:, :nw],
                                mybir.ActivationFunctionType.Copy,
                                bias=0.0, scale=FINV)
                        if mi == dm - 1:
                            dst = out_d[(m - mi) * 128:(m + 1) * 128, nb:nb + nw]
                            if dm > 1:
                                dst = dst.rearrange("(a p) n -> p a n", a=dm)
                            nc.sync.dma_start(dst, o_t[:, :, :nw] if dm > 1
                                              else o_t[:, 0, :nw])
    return nc


def host_prep(contexts, W_e, b_e, W_1, b_1, W_2, b_2):
    contexts = np.asarray(contexts)
    W_ebT = np.asarray(W_e, np.float64).T + np.asarray(b_e, np.float64)
    x_e = W_ebT[contexts.reshape(-1)].reshape(B, CTX * EMB)
    xbar = x_e.mean(axis=0)
    d_e_q = _fp8((x_e - xbar) * SD)
    embT = np.ascontiguousarray(
        d_e_q.reshape(B, KE, 128).transpose(2, 1, 0)).view(np.uint8)

    W1p = np.zeros((1024, CTX * EMB))
    W1p[:HID] = np.asarray(W_1, np.float64)
    w1q = _fp8(W1p.T.reshape(KE, 128, 1024).transpose(1, 0, 2) * SD)
    w1T = np.ascontiguousarray(w1q).view(np.uint8)
    b1p = np.zeros(1024)
    b1p[:HID] = np.asarray(b_1, np.float64)
    b1_eff = b1p + W1p @ xbar
    b1c = np.ascontiguousarray(
        b1_eff.astype(np.float32).reshape(KH, 128).T)
    x0h = np.tanh(b1_eff)
    for i, v in enumerate(CVALS):
        x0h[HID + i] = -v / SD          # dh row becomes exactly v
    x0c = np.ascontiguousarray(
        (-SD * x0h).astype(np.float32).reshape(KH, 128).T)

    VPAD = VC * N_CORES
    W2p = np.zeros((VPAD, K2))
    W2p[:VOCAB, 0:CTX * EMB] = np.asarray(W_2, np.float64)[:, HID:]
    W2p[:VOCAB, CTX * EMB:CTX * EMB + HID] = np.asarray(W_2, np.float64)[:, :HID]
    b2p = np.zeros(VPAD)
    b2p[:VOCAB] = np.asarray(b_2, np.float64)

    x0full = np.concatenate([xbar, x0h[:HID]])
    c = W2p[:, :CTX * EMB + HID] @ x0full + b2p
    W2q = np.zeros((VPAD, K2), F8NP)
    W2q[:, :CTX * EMB + HID] = _fp8(W2p[:, :CTX * EMB + HID] * TW)
    resid = c / FINV
    for i, v in enumerate(CVALS):
        q = _fp8(resid / v)
        W2q[:, CTX * EMB + HID + i] = q
        resid = resid - v * q.astype(np.float64)

    in_maps = []
    for cid in range(N_CORES):
        w2cT = np.ascontiguousarray(
            W2q[cid * VC:(cid + 1) * VC].T.reshape(KE + KH, 128, VC)
            .transpose(1, 0, 2)).view(np.uint8)
        in_maps.append({"embT": embT, "w1T": w1T, "b1c": b1c,
                        "x0c": x0c, "w2T": w2cT})
    return in_maps


_NC_CACHE = {}


def get_nc(reps=1):
    key = ("nc", reps)
    if key not in _NC_CACHE:
        _install_patches()
        _NC_CACHE[key] = _build_kernel(reps)
    return _NC_CACHE[key]


def kernel(contexts, W_e, b_e, W_1, b_1, W_2, b_2):
    nc = get_nc()
    in_maps = host_prep(contexts, W_e, b_e, W_1, b_1, W_2, b_2)
    res = bass_utils.run_bass_kernel_spmd(nc, in_maps, list(range(N_CORES)))
    full = np.concatenate(
        [res.results[c]["out"].astype(np.float32) for c in range(N_CORES)], axis=1)
    return np.ascontiguousarray(full[:, :VOCAB])



# revision 8
# speedup vs baseline: 1.9076x; 1.9076x over previous
"""Bengio-style NNLM forward on 8 Trainium2 NeuronCores (Bass/Tile).

Strategy (vocab-tensor-parallel + fp8 DoubleRow matmuls):
  - W_2/b_2 and the [B, VOCAB] output are sharded across 8 cores along
    vocab (6284 columns each, padded from 50257). The small shared parts
    (embedding rows, fc weights) are replicated.
  - All matmuls run in fp8 e4m3 with MatmulPerfMode.DoubleRow (2 K-rows
    per PE cell, 0.5 cycles/row -> 4x over f32r), f32 PSUM accumulation.
  - Precision via mean compensation: device computes with DEVIATIONS
    d = x - x0 (x0 = host reference point), which are ~6x smaller than x,
    so fp8 quantization noise shrinks proportionally. The exact term
    c = x0 @ W2 + b2 is folded in as 3 constant power-of-2 feature rows
    whose fp8 weights are a greedy residual split of c (error < 2e-6).
      d_e  = fp8(2^12 (x_e - xbar_e))    gathered embeddings, host-side
      h    = tanh(2^-24 (d_e @ W1') + b1')  on ACT, b1' = b1 + xbar_e@W1
      d_h  = fp8(2^12 h - 2^12 x0_h)     second ACT pass (Identity+bias)
      out  = 2^-18 (d @ W2' + c-rows)    bf16 out, scaled copy DVE/ACT
  - fp8 bytes travel as uint8 DRAM params and are bitcast on device.
"""
import json
import numpy as np
import ml_dtypes

import concourse.bass as bass
import concourse.mybir as mybir
import concourse.tile as tile
from concourse import bass_utils, bass2jax

F32 = mybir.dt.float32
F8 = mybir.dt.float8e4
U8 = mybir.dt.uint8
BF16 = mybir.dt.bfloat16
F8NP = ml_dtypes.float8_e4m3
BF16NP = ml_dtypes.bfloat16
DR = mybir.MatmulPerfMode.DoubleRow

EMB = 128
CTX = 8
HID = 1000
VOCAB = 50257
B = 4096
N_CORES = 8
VC = 6284            # per-core vocab shard (8 * 6284 = 50272 >= 50257)
KE, KH = 8, 8        # emb / hidden k-tiles (128 features each)
K2 = (KE + KH) * 128 # 2048 layer-2 contraction

SD = 2.0 ** 12       # deviation scale (d_e, d_h)
TW = 2.0 ** 6        # W2 scale; psum = 2^18 * out
FINV = 2.0 ** -18
L1INV = 2.0 ** -24   # layer-1 psum descale (2^12 * 2^12)
CVALS = (2.0 ** 7, 2.0 ** 3, 2.0 ** -1)   # const-row feature values


# ---------------------------------------------------------------------------
# Workaround for walrus builds that allow only ONE sem-wait per instruction:
# split any multi-wait instruction into wait-only EventSemaphores before it.
# ---------------------------------------------------------------------------
def _split_waits(js):
    for fn in js.get("functions", []):
        for bb in fn.get("blocks", []):
            out = []
            for inst in bb.get("instructions", []):
                si = inst.get("sync_info")
                waits = (si or {}).get("on_wait") or []
                if len(waits) > 1:
                    for k, w in enumerate(waits[:-1]):
                        out.append({
                            "debug": inst.get("debug", 0),
                            "engine": inst["engine"],
                            "ins": [], "outs": [],
                            "name": f"{inst['name']}-wsplit{k}",
                            "opcode": "EventSemaphore",
                            "sync_info": {"on_update": [], "on_wait": [w]},
                        })
                    si["on_wait"] = waits[-1:]
                out.append(inst)
            bb["instructions"] = out
    return js


def _install_patches():
    if getattr(bass_utils.compile_bir_kernel, "_wait_split_patched", False):
        return
    orig = bass_utils.compile_bir_kernel

    def wrapper(bir_json, tmpdir, neff_name="file.neff"):
        js = _split_waits(json.loads(bir_json))
        return orig(json.dumps(js).encode(), tmpdir, neff_name=neff_name)

    wrapper._wait_split_patched = True
    bass_utils.compile_bir_kernel = wrapper
    bass2jax.compile_bir_kernel = wrapper


def _fp8(x):
    return np.asarray(x, np.float32).astype(F8NP)


def _fp8f(x):
    return _fp8(x).astype(np.float64)


def _n_chunks(vc, nmax=512):
    # Balanced widths: a ragged thin tail chunk makes its DR matmuls
    # LDWEIGHTS-bound (~213ns for any nw < ~450); even ~484-wide chunks
    # keep every MM stream-bound.
    n = -(-vc // nmax)
    w = -(-vc // n)
    out, o = [], 0
    while o < vc:
        cw = min(w, vc - o)
        out.append((o, cw))
        o += cw
    return out


def _build_kernel(reps=1, opts=None):
    o = {"psA": 4, "psB": 4, "htp": 2, "w2p": 2, "outp": 4,
         "dh_dve": False, "copy_split": 0, "emb_split": 1,
         "skip_l1": False, "sparse_copy": False, "ht_bf16": True,
         "dma_merge": 4, "dr": True}
    o.update(opts or {})
    nc = bass.Bass()
    embT_d = nc.declare_dram_parameter("embT", [128, KE, B], U8, isOutput=False)
    w1T_d = nc.declare_dram_parameter("w1T", [128, KE, 1024], U8, isOutput=False)
    b1c_d = nc.declare_dram_parameter("b1c", [128, KH], F32, isOutput=False)
    x0c_d = nc.declare_dram_parameter("x0c", [128, KH], F32, isOutput=False)
    w2T_d = nc.declare_dram_parameter("w2T", [128, KE + KH, VC], U8, isOutput=False)
    out_d = nc.declare_dram_parameter("out", [B, VC], BF16, isOutput=True)

    with tile.TileContext(nc) as tc:
        with tc.tile_pool(name="constp", bufs=1) as constp, \
             tc.tile_pool(name="embp", bufs=2) as embp, \
             tc.tile_pool(name="hp", bufs=1) as hp, \
             tc.tile_pool(name="htp", bufs=o["htp"]) as htp, \
             tc.tile_pool(name="w2p", bufs=o["w2p"]) as w2p, \
             tc.tile_pool(name="outp", bufs=o["outp"]) as outp, \
             tc.tile_pool(name="psA", bufs=o["psA"], space="PSUM") as psA, \
             tc.tile_pool(name="psB", bufs=o["psB"], space="PSUM") as psB:

            w1 = constp.tile([128, KE, 1024], F8)
            nc.sync.dma_start(w1[:], w1T_d[:].bitcast(F8))
            b1 = constp.tile([128, KH], F32)
            nc.sync.dma_start(b1[:], b1c_d[:])
            x0 = constp.tile([128, KH], F32)
            nc.sync.dma_start(x0[:], x0c_d[:])

            for _r in range(reps):
                emb = embp.tile([128, KE, B], F8, tag="emb")
                nsp = o["emb_split"]
                for sp in range(nsp):
                    es = slice(sp * (B // nsp), (sp + 1) * (B // nsp))
                    nc.sync.dma_start(emb[:, :, es], embT_d[:, :, es].bitcast(F8))
                dh = emb if o["skip_l1"] else hp.tile([128, KH, B], F8, tag="dh")

                # ---- layer 1: h = tanh(2^-24 ps + b1'); dh = 2^12 h - 2^12 x0_h
                for jt in range(0 if o["skip_l1"] else KH):
                    js = slice(jt * 128, (jt + 1) * 128)
                    for bh in range(B // 512):
                        bs = slice(bh * 512, (bh + 1) * 512)
                        ps = psA.tile([128, 512], F32, tag="psA")
                        if o["dr"]:
                            for k in range(0, KE, 2):
                                nc.tensor.matmul(
                                    ps[:], w1[:, k:k + 2, js],
                                    emb[:, k:k + 2, bs],
                                    start=(k == 0), stop=(k == KE - 2),
                                    perf_mode=DR)
                        else:
                            for k in range(KE):
                                nc.tensor.matmul(
                                    ps[:], w1[:, k, js], emb[:, k, bs],
                                    start=(k == 0), stop=(k == KE - 1))
                        ht = htp.tile([128, 512], BF16 if o["ht_bf16"] else F32,
                                      tag="ht")
                        nc.scalar.activation(
                            ht[:], ps[:], mybir.ActivationFunctionType.Tanh,
                            bias=b1[:, jt:jt + 1], scale=L1INV)
                        if o["dh_dve"]:
                            nc.vector.tensor_scalar(
                                dh[:, jt, bs], ht[:], SD, x0[:, jt:jt + 1],
                                mybir.AluOpType.mult, mybir.AluOpType.add)
                        else:
                            nc.scalar.activation(
                                dh[:, jt, bs], ht[:],
                                mybir.ActivationFunctionType.Identity,
                                bias=x0[:, jt:jt + 1], scale=SD)

                # ---- layer 2: out = 2^-18 (d @ W2' + c-rows)
                for (nb, nw) in _n_chunks(VC):
                    w2 = w2p.tile([128, KE + KH, 512], F8, tag="w2")
                    nc.sync.dma_start(
                        w2[:, :, :nw], w2T_d[:, :, nb:nb + nw].bitcast(F8))
                    for m in range(B // 128):
                        ms = slice(m * 128, (m + 1) * 128)
                        ps = psB.tile([128, 512], F32, tag="psB")
                        if o["dr"]:
                            for k in range(0, KE + KH, 2):
                                lhsT = (emb[:, k:k + 2, ms] if k < KE
                                        else dh[:, k - KE:k - KE + 2, ms])
                                nc.tensor.matmul(
                                    ps[:, :nw], lhsT, w2[:, k:k + 2, :nw],
                                    start=(k == 0), stop=(k == KE + KH - 2),
                                    perf_mode=DR)
                        else:
                            for k in range(KE + KH):
                                lhsT = (emb[:, k, ms] if k < KE
                                        else dh[:, k - KE, ms])
                                nc.tensor.matmul(
                                    ps[:, :nw], lhsT, w2[:, k, :nw],
                                    start=(k == 0), stop=(k == KE + KH - 1))
                        if o["sparse_copy"] and m % 8 != 7:
                            continue
                        dm = o["dma_merge"]
                        mi = m % dm
                        if mi == 0:
                            o_t = outp.tile([128, dm, 512], BF16, tag="o")
                        cs = o["copy_split"]
                        if (cs == 0 or (cs == 2 and m % 2 == 0)
                                or (cs == 4 and m % 3 < 2)):
                            nc.vector.tensor_scalar_mul(
                                o_t[:, mi, :nw], ps[:, :nw], FINV)
                        else:
                            nc.scalar.activation(
                                o_t[:, mi, :nw], ps[:, :nw],
                                mybir.ActivationFunctionType.Copy,
                                bias=0.0, scale=FINV)
                        if mi == dm - 1:
                            dst = out_d[(m - mi) * 128:(m + 1) * 128, nb:nb + nw]
                            if dm > 1:
                                dst = dst.rearrange("(a p) n -> p a n", a=dm)
                            nc.sync.dma_start(dst, o_t[:, :, :nw] if dm > 1
                                              else o_t[:, 0, :nw])
    return nc


def host_prep(contexts, W_e, b_e, W_1, b_1, W_2, b_2):
    contexts = np.asarray(contexts)
    W_ebT = np.asarray(W_e, np.float64).T + np.asarray(b_e, np.float64)
    x_e = W_ebT[contexts.reshape(-1)].reshape(B, CTX * EMB)
    xbar = x_e.mean(axis=0)
    d_e_q = _fp8((x_e - xbar) * SD)
    embT = np.ascontiguousarray(
        d_e_q.reshape(B, KE, 128).transpose(2, 1, 0)).view(np.uint8)

    W1p = np.zeros((1024, CTX * EMB))
    W1p[:HID] = np.asarray(W_1, np.float64)
    w1q = _fp8(W1p.T.reshape(KE, 128, 1024).transpose(1, 0, 2) * SD)
    w1T = np.ascontiguousarray(w1q).view(np.uint8)
    b1p = np.zeros(1024)
    b1p[:HID] = np.asarray(b_1, np.float64)
    b1_eff = b1p + W1p @ xbar
    b1c = np.ascontiguousarray(
        b1_eff.astype(np.float32).reshape(KH, 128).T)
    x0h = np.tanh(b1_eff)
    for i, v in enumerate(CVALS):
        x0h[HID + i] = -v / SD          # dh row becomes exactly v
    x0c = np.ascontiguousarray(
        (-SD * x0h).astype(np.float32).reshape(KH, 128).T)

    VPAD = VC * N_CORES
    W2p = np.zeros((VPAD, K2))
    W2p[:VOCAB, 0:CTX * EMB] = np.asarray(W_2, np.float64)[:, HID:]
    W2p[:VOCAB, CTX * EMB:CTX * EMB + HID] = np.asarray(W_2, np.float64)[:, :HID]
    b2p = np.zeros(VPAD)
    b2p[:VOCAB] = np.asarray(b_2, np.float64)

    x0full = np.concatenate([xbar, x0h[:HID]])
    c = W2p[:, :CTX * EMB + HID] @ x0full + b2p
    W2q = np.zeros((VPAD, K2), F8NP)
    W2q[:, :CTX * EMB + HID] = _fp8(W2p[:, :CTX * EMB + HID] * TW)
    resid = c / FINV
    for i, v in enumerate(CVALS):
        q = _fp8(resid / v)
        W2q[:, CTX * EMB + HID + i] = q
        resid = resid - v * q.astype(np.float64)

    in_maps = []
    for cid in range(N_CORES):
        w2cT = np.ascontiguousarray(
            W2q[cid * VC:(cid + 1) * VC].T.reshape(KE + KH, 128, VC)
            .transpose(1, 0, 2)).view(np.uint8)
        in_maps.append({"embT": embT, "w1T": w1T, "b1c": b1c,
                        "x0c": x0c, "w2T": w2cT})
    return in_maps


_NC_CACHE = {}


def get_nc(reps=1):
    key = ("nc", reps)
    if key not in _NC_CACHE:
        _install_patches()
        _NC_CACHE[key] = _build_kernel(reps)
    return _NC_CACHE[key]


def kernel(contexts, W_e, b_e, W_1, b_1, W_2, b_2):
    nc = get_nc()
    in_maps = host_prep(contexts, W_e, b_e, W_1, b_1, W_2, b_2)
    res = bass_utils.run_bass_kernel_spmd(nc, in_maps, list(range(N_CORES)))
    full = np.concatenate(
        [res.results[c]["out"].astype(np.float32) for c in range(N_CORES)], axis=1)
    return np.ascontiguousarray(full[:, :VOCAB])



# revision 15
# speedup vs baseline: 1.9774x; 1.0366x over previous
"""Bengio-style NNLM forward on 8 Trainium2 NeuronCores (Bass/Tile).

Strategy (vocab-tensor-parallel + fp8 DoubleRow matmuls):
  - W_2/b_2 and the [B, VOCAB] output are sharded across 8 cores along
    vocab (6284 columns each, padded from 50257). The small shared parts
    (embedding rows, fc weights) are replicated.
  - All matmuls run in fp8 e4m3 with MatmulPerfMode.DoubleRow (2 K-rows
    per PE cell, 0.5 cycles/row -> 4x over f32r), f32 PSUM accumulation.
  - Precision via mean compensation: device computes with DEVIATIONS
    d = x - x0 (x0 = host reference point), which are ~6x smaller than x,
    so fp8 quantization noise shrinks proportionally. The exact term
    c = x0 @ W2 + b2 is folded in as 3 constant power-of-2 feature rows
    whose fp8 weights are a greedy residual split of c (error < 2e-6).
      d_e  = fp8(2^12 (x_e - xbar_e))    gathered embeddings, host-side
      h    = tanh(2^-24 (d_e @ W1') + b1')  on ACT, b1' = b1 + xbar_e@W1
      d_h  = fp8(2^12 h - 2^12 x0_h)     second ACT pass (Identity+bias)
      out  = 2^-18 (d @ W2' + c-rows)    bf16 out, scaled copy DVE/ACT
  - fp8 bytes travel as uint8 DRAM params and are bitcast on device.
"""
import json
import numpy as np
import ml_dtypes

import concourse.bass as bass
import concourse.mybir as mybir
import concourse.tile as tile
from concourse import bass_utils, bass2jax

F32 = mybir.dt.float32
F8 = mybir.dt.float8e4
U8 = mybir.dt.uint8
BF16 = mybir.dt.bfloat16
F8NP = ml_dtypes.float8_e4m3
BF16NP = ml_dtypes.bfloat16
DR = mybir.MatmulPerfMode.DoubleRow

EMB = 128
CTX = 8
HID = 1000
VOCAB = 50257
B = 4096
N_CORES = 8
VC = 6284            # per-core vocab shard (8 * 6284 = 50272 >= 50257)
KE, KH = 8, 8        # emb / hidden k-tiles (128 features each)
K2 = (KE + KH) * 128 # 2048 layer-2 contraction

SD = 2.0 ** 12       # deviation scale (d_e, d_h)
TW = 2.0 ** 6        # W2 scale; psum = 2^18 * out
FINV = 2.0 ** -18
L1INV = 2.0 ** -24   # layer-1 psum descale (2^12 * 2^12)
CVALS = (2.0 ** 7, 2.0 ** 3, 2.0 ** -1)   # const-row feature values


# ---------------------------------------------------------------------------
# Workaround for walrus builds that allow only ONE sem-wait per instruction:
# split any multi-wait instruction into wait-only EventSemaphores before it.
# ---------------------------------------------------------------------------
def _split_waits(js):
    for fn in js.get("functions", []):
        for bb in fn.get("blocks", []):
            out = []
            for inst in bb.get("instructions", []):
                si = inst.get("sync_info")
                waits = (si or {}).get("on_wait") or []
                if len(waits) > 1:
                    for k, w in enumerate(waits[:-1]):
                        out.append({
                            "debug": inst.get("debug", 0),
                            "engine": inst["engine"],
                            "ins": [], "outs": [],
                            "name": f"{inst['name']}-wsplit{k}",
                            "opcode": "EventSemaphore",
                            "sync_info": {"on_update": [], "on_wait": [w]},
                        })
                    si["on_wait"] = waits[-1:]
                out.append(inst)
            bb["instructions"] = out
    return js


def _install_patches():
    if getattr(bass_utils.compile_bir_kernel, "_wait_split_patched", False):
        return
    orig = bass_utils.compile_bir_kernel

    def wrapper(bir_json, tmpdir, neff_name="file.neff"):
        js = _split_waits(json.loads(bir_json))
        return orig(json.dumps(js).encode(), tmpdir, neff_name=neff_name)

    wrapper._wait_split_patched = True
    bass_utils.compile_bir_kernel = wrapper
    bass2jax.compile_bir_kernel = wrapper


def _fp8(x):
    return np.asarray(x, np.float32).astype(F8NP)


def _fp8f(x):
    return _fp8(x).astype(np.float64)


def _n_chunks(vc, nmax=512, even=True):
    # Balanced widths: a ragged thin tail chunk makes its DR matmuls
    # LDWEIGHTS-bound (~213ns for any nw < ~450); even ~484-wide chunks
    # keep every MM stream-bound.
    n = -(-vc // nmax)
    w = -(-vc // n) if even else nmax
    out, o = [], 0
    while o < vc:
        cw = min(w, vc - o)
        out.append((o, cw))
        o += cw
    return out


def _build_kernel(reps=1, opts=None):
    o = {"psA": 4, "psB": 4, "htp": 2, "w2p": 2, "outp": 4,
         "dh_dve": False, "copy_split": 4, "emb_split": 1,
         "skip_l1": False, "sparse_copy": False, "ht_bf16": True,
         "dma_merge": 4, "dr": True, "even_chunks": True, "l1_wide": False}
    o.update(opts or {})
    nc = bass.Bass()
    embT_d = nc.declare_dram_parameter("embT", [128, KE, B], U8, isOutput=False)
    w1T_d = nc.declare_dram_parameter("w1T", [128, KE, 1024], U8, isOutput=False)
    b1c_d = nc.declare_dram_parameter("b1c", [128, KH], F32, isOutput=False)
    x0c_d = nc.declare_dram_parameter("x0c", [128, KH], F32, isOutput=False)
    w2T_d = nc.declare_dram_parameter("w2T", [128, KE + KH, VC], U8, isOutput=False)
    out_d = nc.declare_dram_parameter("out", [B, VC], BF16, isOutput=True)

    with tile.TileContext(nc) as tc:
        with tc.tile_pool(name="constp", bufs=1) as constp, \
             tc.tile_pool(name="embp", bufs=2) as embp, \
             tc.tile_pool(name="hp", bufs=1) as hp, \
             tc.tile_pool(name="htp", bufs=o["htp"]) as htp, \
             tc.tile_pool(name="w2p", bufs=o["w2p"]) as w2p, \
             tc.tile_pool(name="outp", bufs=o["outp"]) as outp, \
             tc.tile_pool(name="psA", bufs=o["psA"], space="PSUM") as psA, \
             tc.tile_pool(name="psB", bufs=o["psB"], space="PSUM") as psB:

            w1 = constp.tile([128, KE, 1024], F8)
            nc.sync.dma_start(w1[:], w1T_d[:].bitcast(F8))
            b1 = constp.tile([128, KH], F32)
            nc.sync.dma_start(b1[:], b1c_d[:])
            x0 = constp.tile([128, KH], F32)
            nc.sync.dma_start(x0[:], x0c_d[:])

            for _r in range(reps):
                emb = embp.tile([128, KE, B], F8, tag="emb")
                nsp = o["emb_split"]
                for sp in range(nsp):
                    es = slice(sp * (B // nsp), (sp + 1) * (B // nsp))
                    nc.sync.dma_start(emb[:, :, es], embT_d[:, :, es].bitcast(F8))
                dh = emb if o["skip_l1"] else hp.tile([128, KH, B], F8, tag="dh")

                # ---- layer 1: h = tanh(2^-24 ps + b1'); dh = 2^12 h - 2^12 x0_h
                for jt in range(0 if o["skip_l1"] else (KH if o["l1_wide"] else 0)):
                    js = slice(jt * 128, (jt + 1) * 128)
                    for bh in range(B // 1024):
                        bs = slice(bh * 1024, (bh + 1) * 1024)
                        ps = psA.tile([128, 1024], F32, tag="psA")
                        for half in range(2):
                            hs = slice(half * 512, half * 512 + 512)
                            bsh = slice(bh * 1024 + half * 512,
                                        bh * 1024 + half * 512 + 512)
                            if o["dr"]:
                                for k in range(0, KE, 2):
                                    nc.tensor.matmul(
                                        ps[:, hs], w1[:, k:k + 2, js],
                                        emb[:, k:k + 2, bsh],
                                        start=(k == 0), stop=(k == KE - 2),
                                        perf_mode=DR)
                            else:
                                for k in range(KE):
                                    nc.tensor.matmul(
                                        ps[:, hs], w1[:, k, js], emb[:, k, bsh],
                                        start=(k == 0), stop=(k == KE - 1))
                        ht = htp.tile([128, 1024], BF16 if o["ht_bf16"] else F32,
                                      tag="ht")
                        nc.scalar.activation(
                            ht[:], ps[:], mybir.ActivationFunctionType.Tanh,
                            bias=b1[:, jt:jt + 1], scale=L1INV)
                        nc.scalar.activation(
                            dh[:, jt, bs], ht[:],
                            mybir.ActivationFunctionType.Identity,
                            bias=x0[:, jt:jt + 1], scale=SD)

                for jt in range(0 if (o["skip_l1"] or o["l1_wide"]) else KH):
                    js = slice(jt * 128, (jt + 1) * 128)
                    for bh in range(B // 512):
                        bs = slice(bh * 512, (bh + 1) * 512)
                        ps = psA.tile([128, 512], F32, tag="psA")
                        if o["dr"]:
                            for k in range(0, KE, 2):
                                nc.tensor.matmul(
                                    ps[:], w1[:, k:k + 2, js],
                                    emb[:, k:k + 2, bs],
                                    start=(k == 0), stop=(k == KE - 2),
                                    perf_mode=DR)
                        else:
                            for k in range(KE):
                                nc.tensor.matmul(
                                    ps[:], w1[:, k, js], emb[:, k, bs],
                                    start=(k == 0), stop=(k == KE - 1))
                        ht = htp.tile([128, 512], BF16 if o["ht_bf16"] else F32,
                                      tag="ht")
                        nc.scalar.activation(
                            ht[:], ps[:], mybir.ActivationFunctionType.Tanh,
                            bias=b1[:, jt:jt + 1], scale=L1INV)
                        if o["dh_dve"]:
                            nc.vector.tensor_scalar(
                                dh[:, jt, bs], ht[:], SD, x0[:, jt:jt + 1],
                                mybir.AluOpType.mult, mybir.AluOpType.add)
                        else:
                            nc.scalar.activation(
                                dh[:, jt, bs], ht[:],
                                mybir.ActivationFunctionType.Identity,
                                bias=x0[:, jt:jt + 1], scale=SD)

                # ---- layer 2: out = 2^-18 (d @ W2' + c-rows)
                for (nb, nw) in _n_chunks(VC, even=o["even_chunks"]):
                    w2 = w2p.tile([128, KE + KH, 512], F8, tag="w2")
                    nc.sync.dma_start(
                        w2[:, :, :nw], w2T_d[:, :, nb:nb + nw].bitcast(F8))
                    for m in range(B // 128):
                        ms = slice(m * 128, (m + 1) * 128)
                        ps = psB.tile([128, 512], F32, tag="psB")
                        if o["dr"]:
                            for k in range(0, KE + KH, 2):
                                lhsT = (emb[:, k:k + 2, ms] if k < KE
                                        else dh[:, k - KE:k - KE + 2, ms])
                                nc.tensor.matmul(
                                    ps[:, :nw], lhsT, w2[:, k:k + 2, :nw],
                                    start=(k == 0), stop=(k == KE + KH - 2),
                                    perf_mode=DR)
                        else:
                            for k in range(KE + KH):
                                lhsT = (emb[:, k, ms] if k < KE
                                        else dh[:, k - KE, ms])
                                nc.tensor.matmul(
                                    ps[:, :nw], lhsT, w2[:, k, :nw],
                                    start=(k == 0), stop=(k == KE + KH - 1))
                        if o["sparse_copy"] and m % 8 != 7:
                            continue
                        dm = o["dma_merge"]
                        mi = m % dm
                        if mi == 0:
                            o_t = outp.tile([128, dm, 512], BF16, tag="o")
                        cs = o["copy_split"]
                        if (cs == 0 or (cs == 2 and m % 2 == 0)
                                or (cs == 4 and m % 3 < 2)):
                            nc.vector.tensor_scalar_mul(
                                o_t[:, mi, :nw], ps[:, :nw], FINV)
                        else:
                            nc.scalar.activation(
                                o_t[:, mi, :nw], ps[:, :nw],
                                mybir.ActivationFunctionType.Copy,
                                bias=0.0, scale=FINV)
                        if mi == dm - 1:
                            dst = out_d[(m - mi) * 128:(m + 1) * 128, nb:nb + nw]
                            if dm > 1:
                                dst = dst.rearrange("(a p) n -> p a n", a=dm)
                            nc.sync.dma_start(dst, o_t[:, :, :nw] if dm > 1
                                              else o_t[:, 0, :nw])
    return nc


def host_prep(contexts, W_e, b_e, W_1, b_1, W_2, b_2):
    contexts = np.asarray(contexts)
    W_ebT = np.asarray(W_e, np.float64).T + np.asarray(b_e, np.float64)
    x_e = W_ebT[contexts.reshape(-1)].reshape(B, CTX * EMB)
    xbar = x_e.mean(axis=0)
    d_e_q = _fp8((x_e - xbar) * SD)
    embT = np.ascontiguousarray(
        d_e_q.reshape(B, KE, 128).transpose(2, 1, 0)).view(np.uint8)

    W1p = np.zeros((1024, CTX * EMB))
    W1p[:HID] = np.asarray(W_1, np.float64)
    w1q = _fp8(W1p.T.reshape(KE, 128, 1024).transpose(1, 0, 2) * SD)
    w1T = np.ascontiguousarray(w1q).view(np.uint8)
    b1p = np.zeros(1024)
    b1p[:HID] = np.asarray(b_1, np.float64)
    b1_eff = b1p + W1p @ xbar
    b1c = np.ascontiguousarray(
        b1_eff.astype(np.float32).reshape(KH, 128).T)
    x0h = np.tanh(b1_eff)
    for i, v in enumerate(CVALS):
        x0h[HID + i] = -v / SD          # dh row becomes exactly v
    x0c = np.ascontiguousarray(
        (-SD * x0h).astype(np.float32).reshape(KH, 128).T)

    VPAD = VC * N_CORES
    W2p = np.zeros((VPAD, K2))
    W2p[:VOCAB, 0:CTX * EMB] = np.asarray(W_2, np.float64)[:, HID:]
    W2p[:VOCAB, CTX * EMB:CTX * EMB + HID] = np.asarray(W_2, np.float64)[:, :HID]
    b2p = np.zeros(VPAD)
    b2p[:VOCAB] = np.asarray(b_2, np.float64)

    x0full = np.concatenate([xbar, x0h[:HID]])
    c = W2p[:, :CTX * EMB + HID] @ x0full + b2p
    W2q = np.zeros((VPAD, K2), F8NP)
    W2q[:, :CTX * EMB + HID] = _fp8(W2p[:, :CTX * EMB + HID] * TW)
    resid = c / FINV
    for i, v in enumerate(CVALS):
        q = _fp8(resid / v)
        W2q[:, CTX * EMB + HID + i] = q
        resid = resid - v * q.astype(np.float64)

    in_maps = []
    for cid in range(N_CORES):
        w2cT = np.ascontiguousarray(
            W2q[cid * VC:(cid + 1) * VC].T.reshape(KE + KH, 128, VC)
            .transpose(1, 0, 2)).view(np.uint8)
        in_maps.append({"embT": embT, "w1T": w1T, "b1c": b1c,
                        "x0c": x0c, "w2T": w2cT})
    return in_maps


_NC_CACHE = {}


def get_nc(reps=1, opts=None):
    key = ("nc", reps, tuple(sorted((opts or {}).items())))
    if key not in _NC_CACHE:
        _install_patches()
        _NC_CACHE[key] = _build_kernel(reps, opts)
    return _NC_CACHE[key]


def kernel(contexts, W_e, b_e, W_1, b_1, W_2, b_2):
    nc = get_nc()
    in_maps = host_prep(contexts, W_e, b_e, W_1, b_1, W_2, b_2)
    res = bass_utils.run_bass_kernel_spmd(nc, in_maps, list(range(N_CORES)))
    full = np.concatenate(
        [res.results[c]["out"].astype(np.float32) for c in range(N_CORES)], axis=1)
    return np.ascontiguousarray(full[:, :VOCAB])

